# revision 15
# baseline (speedup 1.0000x reference)
"""Trainium2 Bass kernel for nn_MeaMDensity22 (gnn_message_passing).

Strategy (data-parallel over molecules, 2 molecules per NeuronCore):
  * Host sorts each molecule's 8192 pairs by center atom into a grid
    [K_pad rows, 128 atom-columns] (K_pad = max neighbor count, rounded to 32).
    Pairs of atom `a` occupy column `a`; padding slots are masked to zero.
  * On device, the segment-sum over pairs becomes one small PE matmul per
    atom column:  sumw_a^T [32,12] = Gauss_a[K,32].T @ Ang_a[K,12]  -- the
    angular-outer-gaussian accumulation happens inside the systolic array,
    so the (pairs x 12 x 32) `worb` tensor is never materialized.
  * Center-atom data is broadcast along the free dim (per-column constants)
    via a single K=1 ones-matmul into PSUM; per-pair elementwise chain
    (dist, cutoff, gaussians, angular) runs on DVE/ACT over big tiles.
  * Activation table sets are phase-grouped (Rsqrt -> Sin -> Exp/Square).

Host-side work is limited to index-derived preprocessing (sort/permute of
pair-indexed arrays and staging layouts) and the j-endpoint coordinate
permutation into the grid; all arithmetic runs on device.
"""

import math
import os
import sys

import numpy as np

sys.path.insert(0, "/opt/trn_rl_repo")

A = 128          # atoms per molecule
G = 32           # gaussians
E = 3            # species
LDIM = 12        # angular rows (3 + 9)
CUTOFF = 5.0
NCORES = 8
NMOL = 2         # molecules per core
PI = math.pi


def _prep_molecule(coords_b, shifts_b, idx_b, KP):
    """Build sorted center-grid arrays for one molecule.

    Returns sh_g [KP,A,3], cj_g [KP,A,3], mask_g [KP,A] float32.
    """
    i = np.asarray(idx_b[0], np.int64)
    j = np.asarray(idx_b[1], np.int64)
    order = np.argsort(i, kind="stable")
    i_s = i[order]
    counts = np.bincount(i, minlength=A)
    starts = np.zeros(A, np.int64)
    starts[1:] = np.cumsum(counts)[:-1]
    rows = np.arange(i.shape[0], dtype=np.int64) - starts[i_s]
    cols = i_s

    valid = np.all(shifts_b > -1e9, axis=1).astype(np.float32)

    sh_g = np.zeros((KP, A, 3), np.float32)
    cj_g = np.zeros((KP, A, 3), np.float32)
    mask_g = np.zeros((KP, A), np.float32)
    sh_g[rows, cols] = shifts_b[order]
    cj_g[rows, cols] = coords_b[j[order]]
    mask_g[rows, cols] = valid[order]
    return sh_g, cj_g, mask_g


def _build_program(KP, uniform_w):
    """Build the per-core Bass program (same program for all 8 cores)."""
    import concourse.bass as bass
    import concourse.bacc as bacc
    import concourse.tile as tile
    from concourse import mybir

    f32 = mybir.dt.float32
    AF = mybir.ActivationFunctionType
    OP = mybir.AluOpType
    X = mybir.AxisListType.X

    nc = bacc.Bacc("TRN2")

    geo_d = nc.dram_tensor("geo", [NMOL, KP, A * 6], f32, kind="ExternalInput")
    mask_d = nc.dram_tensor("mask", [NMOL, KP, A], f32, kind="ExternalInput")
    cart_d = nc.dram_tensor("cart", [NMOL, 1, A * 3], f32, kind="ExternalInput")
    offs_d = nc.dram_tensor("offs", [1, E * G], f32, kind="ExternalInput")
    scf_d = nc.dram_tensor("scf", [NMOL, 1, A], f32, kind="ExternalInput")
    out_d = nc.dram_tensor("dens", [NMOL, 2 * A, G], f32, kind="ExternalOutput")

    with tile.TileContext(nc) as tc:
        import contextlib
        ctx = contextlib.ExitStack()
        with ctx:
            singles = ctx.enter_context(tc.tile_pool(name="singles", bufs=1))
            work = ctx.enter_context(tc.tile_pool(name="work", bufs=2))
            big = ctx.enter_context(tc.tile_pool(name="big", bufs=2))
            psum = ctx.enter_context(tc.tile_pool(name="psum", bufs=1, space="PSUM"))
            psum_sw = ctx.enter_context(
                tc.tile_pool(name="psum_sw", bufs=2, space="PSUM")
            )

            # ---- constants ----
            ones_row = singles.tile([1, 128], f32)
            nc.vector.memset(ones_row, 1.0)

            offs_t = singles.tile([1, E * G], f32)
            nc.sync.dma_start(out=offs_t, in_=offs_d[:])
            # w = -0.5 / offs^2
            winv = singles.tile([1, E * G], f32)
            nc.vector.reciprocal(winv[:], offs_t[:])
            w2 = singles.tile([1, E * G], f32)
            nc.vector.tensor_tensor(out=w2[:], in0=winv[:], in1=winv[:], op=OP.mult)
            wf = singles.tile([1, E * G], f32)
            nc.vector.tensor_scalar(
                out=wf[:], in0=w2[:], scalar1=-0.5, scalar2=None, op0=OP.mult
            )

            identity = singles.tile([128, 128], f32)
            from concourse.masks import make_identity
            make_identity(nc, identity[:])

            halfpi = singles.tile([128, 1], f32)
            nc.vector.memset(halfpi, PI / 2.0)
            piC = singles.tile([128, 1], f32)
            nc.vector.memset(piC, -PI / CUTOFF)

            # per-molecule state kept across phases
            st = [dict() for _ in range(NMOL)]

            # ================= phase 1: geometry -> d2 (both molecules) ======
            for m in range(NMOL):
                geo_t = big.tile([KP, A, 6], f32, tag="geo")
                mask_t = work.tile([KP, A], f32, tag="mask")
                cart_t = work.tile([1, A * 3], f32, tag="cart")
                nc.sync.dma_start(out=geo_t, in_=geo_d[m].rearrange("k (a c) -> k a c", c=6))
                nc.sync.dma_start(out=mask_t, in_=mask_d[m])
                nc.sync.dma_start(out=cart_t, in_=cart_d[m])
                sh_t = geo_t[:, :, 0:3]
                cj_t = geo_t[:, :, 3:6]

                # ci broadcast: [KP, A*3] = ones[1,KP].T @ cart[1, A*3]
                ci_ps = psum.tile([KP, A * 3], f32, tag="ci")
                nc.tensor.matmul(
                    ci_ps[:], ones_row[:1, :KP], cart_t[:], start=True, stop=True
                )

                # tiny DVE "observer" copies: advance the DVE vector clock past
                # the DMAs and the PE broadcast so the big TTs below need at
                # most 2 sem waits (TT wait-slot capacity).
                obs = work.tile([1, 4], f32, tag="obs")
                nc.vector.tensor_copy(out=obs[:, 0:1], in_=geo_t[0:1, 0, 0:1])
                nc.vector.tensor_copy(out=obs[:, 1:2], in_=mask_t[0:1, 0:1])
                nc.vector.tensor_copy(out=obs[:, 2:3], in_=ci_ps[0:1, 0:1])

                # dvec = ci - (cj - sh)
                dvec = big.tile([KP, A, 3], f32, tag="dvec")
                nc.vector.tensor_tensor(out=dvec[:], in0=cj_t, in1=sh_t, op=OP.subtract)
                nc.vector.tensor_tensor(
                    out=dvec[:],
                    in0=ci_ps[:].rearrange("k (a c) -> k a c", c=3),
                    in1=dvec[:],
                    op=OP.subtract,
                )

                sq = big.tile([KP, A, 3], f32, tag="sq")
                nc.vector.tensor_tensor(out=sq[:], in0=dvec[:], in1=dvec[:], op=OP.mult)
                d2 = work.tile([KP, A], f32, tag="d2")
                nc.vector.reduce_sum(d2[:].unsqueeze(2), sq[:], axis=X)
                st[m].update(dvec=dvec, d2=d2, mask=mask_t)

            # ================= phase 2: Sqrt set (rsq = sqrt(1/d2)) ==========
            for m in range(NMOL):
                ri2 = work.tile([KP, A], f32, tag="ri2")
                nc.vector.reciprocal(ri2[:], st[m]["d2"][:])
                rsq = work.tile([KP, A], f32, tag="rsq")
                nc.scalar.activation(rsq[:], ri2[:], AF.Sqrt)
                st[m]["rsq"] = rsq

            # ================= phase 3: Sin set (cutoff cosine) ==============
            for m in range(NMOL):
                dist = work.tile([KP, A], f32, tag="dist")
                nc.vector.tensor_tensor(
                    out=dist[:], in0=st[m]["d2"][:], in1=st[m]["rsq"][:], op=OP.mult
                )
                dmin = work.tile([KP, A], f32, tag="dmin")
                nc.vector.tensor_scalar(
                    out=dmin[:], in0=dist[:], scalar1=CUTOFF, scalar2=None, op0=OP.min
                )
                cosv = work.tile([KP, A], f32, tag="cosv")
                nc.scalar.activation(
                    cosv[:], dmin[:], AF.Sin,
                    bias=halfpi[:KP, :], scale=piC[:KP, :],
                )
                # cutm = (0.5*cos + 0.5) * mask
                cutm = work.tile([KP, A], f32, tag="cutm")
                nc.vector.tensor_scalar(
                    out=cutm[:], in0=cosv[:], scalar1=0.5, scalar2=0.5,
                    op0=OP.mult, op1=OP.add,
                )
                nc.vector.tensor_tensor(
                    out=cutm[:], in0=cutm[:], in1=st[m]["mask"][:], op=OP.mult
                )
                st[m]["cutm"] = cutm

            # ================= phase 4: angular ==============================
            for m in range(NMOL):
                dvec = st[m]["dvec"]
                rsq = st[m]["rsq"]
                cutm = st[m]["cutm"]
                unit = big.tile([KP, A, 3], f32, tag="unit")
                nc.vector.tensor_tensor(
                    out=unit[:],
                    in0=dvec[:],
                    in1=rsq[:].unsqueeze(2).broadcast_to([KP, A, 3]),
                    op=OP.mult,
                )
                ang = big.tile([KP, A, LDIM], f32, tag="ang")
                nc.vector.tensor_tensor(
                    out=ang[:, :, 0:3],
                    in0=unit[:],
                    in1=cutm[:].unsqueeze(2).broadcast_to([KP, A, 3]),
                    op=OP.mult,
                )
                # ang9[i,j] = unit_i * ang3_j
                nc.vector.tensor_tensor(
                    out=ang[:, :, 3:12].rearrange("k a (i j) -> k a i j", i=3),
                    in0=unit[:].unsqueeze(3).broadcast_to([KP, A, 3, 3]),
                    in1=ang[:, :, 0:3].unsqueeze(2).broadcast_to([KP, A, 3, 3]),
                    op=OP.mult,
                )
                st[m]["ang"] = ang

            # ================= phase 5: gaussian arg ==========================
            # wbc3[k, s, g] = w[s, g] broadcast over partitions
            wbc_ps = psum.tile([KP, E * G], f32, tag="wbc")
            nc.tensor.matmul(wbc_ps[:], ones_row[:1, :KP], wf[:], start=True, stop=True)
            wbc = singles.tile([KP, E, G], f32)
            nc.scalar.copy(wbc[:], wbc_ps[:].rearrange("k (s g) -> k s g", g=G))
            obs_w = singles.tile([1, 1], f32)
            nc.vector.tensor_copy(out=obs_w[:], in_=wbc[0:1, 0, 0:1])

            for m in range(NMOL):
                d2 = st[m]["d2"]
                targ = big.tile([KP, A, G], f32, tag="targ")
                if uniform_w:
                    nc.vector.tensor_tensor(
                        out=targ[:],
                        in0=d2[:].unsqueeze(2).broadcast_to([KP, A, G]),
                        in1=wbc[:, 0:1, :].broadcast_to([KP, A, G]),
                        op=OP.mult,
                    )
                else:
                    # general species path: wpair by select on species scalars
                    scf_t = work.tile([1, A], f32, tag="scf")
                    nc.sync.dma_start(out=scf_t, in_=scf_d[m])
                    sc_ps = psum.tile([KP, A], f32, tag="ci")
                    nc.tensor.matmul(
                        sc_ps[:], ones_row[:1, :KP], scf_t[:], start=True, stop=True
                    )
                    wpair = big.tile([KP, A, G], f32, tag="wpair")
                    m1 = work.tile([KP, A], f32, tag="m1")
                    nc.vector.tensor_scalar(
                        out=m1[:], in0=sc_ps[:], scalar1=1.0, scalar2=None,
                        op0=OP.is_equal,
                    )
                    m2 = work.tile([KP, A], f32, tag="m2")
                    nc.vector.tensor_scalar(
                        out=m2[:], in0=sc_ps[:], scalar1=2.0, scalar2=None,
                        op0=OP.is_equal,
                    )
                    nc.vector.select(
                        out=wpair[:],
                        mask=m1[:].unsqueeze(2).broadcast_to([KP, A, G]),
                        on_true=wbc[:, 1:2, :].broadcast_to([KP, A, G]),
                        on_false=wbc[:, 0:1, :].broadcast_to([KP, A, G]),
                    )
                    nc.vector.select(
                        out=wpair[:],
                        mask=m2[:].unsqueeze(2).broadcast_to([KP, A, G]),
                        on_true=wbc[:, 2:3, :].broadcast_to([KP, A, G]),
                        on_false=wpair[:],
                    )
                    nc.vector.tensor_tensor(
                        out=targ[:],
                        in0=d2[:].unsqueeze(2).broadcast_to([KP, A, G]),
                        in1=wpair[:],
                        op=OP.mult,
                    )
                st[m]["targ"] = targ

            # ================= phase 6: Exp + per-atom matmuls + Square ======
            for m in range(NMOL):
                gauss = big.tile([KP, A, G], f32, tag="gauss")
                nc.scalar.activation(gauss[:], st[m]["targ"][:], AF.Exp)
                ang = st[m]["ang"]

                # 4 psum banks, each 32 atoms: sumw_T[a] = [32, 12]
                dens_pre = work.tile([32, 2, A], f32, tag="dens_pre")
                for bank in range(4):
                    sw_ps = psum_sw.tile([32, 32 * LDIM], f32, tag="sw")
                    for ai in range(32):
                        a = bank * 32 + ai
                        nc.tensor.matmul(
                            sw_ps[:, ai * LDIM:(ai + 1) * LDIM],
                            gauss[:, a, :],
                            ang[:, a, :],
                            start=True,
                            stop=True,
                        )
                    sq_sw = work.tile([32, 32 * LDIM], f32, tag="sq_sw")
                    nc.scalar.activation(sq_sw[:], sw_ps[:], AF.Square)
                    # reduce l-slices: order0 = l 0:3, order1 = l 3:12
                    v = sq_sw[:].rearrange("g (a l) -> g a l", l=LDIM)
                    nc.vector.reduce_sum(
                        dens_pre[:, 0, bank * 32:(bank + 1) * 32].unsqueeze(2),
                        v[:, :, 0:3],
                        axis=X,
                    )
                    nc.vector.reduce_sum(
                        dens_pre[:, 1, bank * 32:(bank + 1) * 32].unsqueeze(2),
                        v[:, :, 3:12],
                        axis=X,
                    )

                # transpose [32, 2*A] -> two [128, 32] chunks (rows = o*A + a)
                dens_sb = work.tile([128, 2, G], f32, tag="dens_sb")
                dp = dens_pre[:].rearrange("g o a -> g (o a)")
                for half in range(2):
                    tp_ps = psum.tile([128, 32], f32, tag="tp")
                    nc.tensor.transpose(
                        tp_ps[:],
                        dp[:, half * 128:(half + 1) * 128],
                        identity[:32, :32],
                    )
                    nc.scalar.copy(dens_sb[:, half, :], tp_ps[:])
                    nc.sync.dma_start(
                        out=out_d[m][half * 128:(half + 1) * 128, :],
                        in_=dens_sb[:, half, :],
                    )

    nc.compile()
    return nc


_PROGRAM_CACHE = {}


def _get_program(KP, uniform_w):
    key = (KP, uniform_w)
    if key not in _PROGRAM_CACHE:
        _PROGRAM_CACHE[key] = _build_program(KP, uniform_w)
    return _PROGRAM_CACHE[key]


def kernel(coordinates, shifts, ang_offsets, atom_index, species, numatoms):
    from concourse.bass_utils import run_bass_kernel_spmd

    coordinates = np.asarray(coordinates, np.float32)
    shifts = np.asarray(shifts, np.float32)
    ang_offsets = np.asarray(ang_offsets, np.float32)
    atom_index = np.asarray(atom_index)
    species = np.asarray(species)

    B, A_, _ = coordinates.shape
    assert A_ == A and B == NCORES * NMOL

    # global K_pad (same program on all cores)
    KP = 32
    for b in range(B):
        cnts = np.bincount(np.asarray(atom_index[b, 0], np.int64), minlength=A)
        KP = max(KP, int(cnts.max()))
    KP = min(128, int(math.ceil(KP / 32.0) * 32))
    uniform_w = bool(np.all(ang_offsets == ang_offsets[0:1]))

    nc = _get_program(KP, uniform_w)

    in_maps = []
    for c in range(NCORES):
        geo_all = np.zeros((NMOL, KP, A * 6), np.float32)
        mask_all = np.zeros((NMOL, KP, A), np.float32)
        cart_all = np.zeros((NMOL, 1, A * 3), np.float32)
        scf_all = np.zeros((NMOL, 1, A), np.float32)
        for m in range(NMOL):
            b = c * NMOL + m
            sh_g, cj_g, mask_g = _prep_molecule(
                coordinates[b], shifts[b], atom_index[b], KP
            )
            geo_all[m] = np.concatenate([sh_g, cj_g], axis=2).reshape(KP, A * 6)
            mask_all[m] = mask_g
            cart_all[m, 0] = coordinates[b].reshape(-1)
            scf_all[m, 0] = np.asarray(species[b * A:(b + 1) * A], np.float32)
        in_maps.append(
            {
                "geo": geo_all,
                "mask": mask_all,
                "cart": cart_all,
                "offs": ang_offsets.reshape(1, E * G).astype(np.float32),
                "scf": scf_all,
            }
        )

    trace = bool(int(os.environ.get("KERNEL_TRACE", "0")))
    res = run_bass_kernel_spmd(
        nc, in_maps, core_ids=list(range(NCORES)), trace=trace
    )
    if trace and res.exec_time_ns is not None:
        print(f"HW exec time: {res.exec_time_ns} ns")
        if res.instructions_and_trace is not None:
            print(f"trace: {res.instructions_and_trace[1]}")

    out = np.zeros((B * A, 2 * G), np.float32)
    for c in range(NCORES):
        dens = res.results[c]["dens"]  # [NMOL, 2A, G]
        for m in range(NMOL):
            b = c * NMOL + m
            d = dens[m].reshape(2, A, G)  # rows (o, a)
            out[b * A:(b + 1) * A, 0:G] = d[0]
            out[b * A:(b + 1) * A, G:2 * G] = d[1]
    return out
